# revision 5
# baseline (speedup 1.0000x reference)
"""AngleLoss (HANDS17 bone-angle loss) on 8 TRN2 NeuronCores.

Math (per batch element b, bone pair (i0, i1)):
    v1 = pred[b, i0, :2] - pred[b, i1, :2]
    v2 = gt[b, i0, :2]   - gt[b, i1, :2]
    t  = |v1 . v2| / (|v1| |v2|)
    loss = mean over (b, pair) of (1 - t)

Strategy: pure data parallel over the batch; each core streams its
65536-element shard (33 MB f32) through SBUF, which pins the roofline
at the ~358 GB/s per-core DMA rate (~96 us). The elementwise pipeline
is spread over THREE engines (DVE, ACT, Pool) so no engine exceeds the
DMA floor:

  - batch-innermost bf16 layout u[t(2), xy(2), joint(21), c(C)] makes
    every post-cast operand a C-long unit-stride bf16 run -> DVE 2x
    mode, and turns the xy pair-reductions into adds of two contiguous
    halves instead of stride-2 gathers.
  - ACT: strided f32->bf16 cast of pred, Square, and Rsqrt(den+eps)
    (single table lookup instead of Ln+Exp; tolerance is 2e-2).
  - DVE: cast of gt, the four bone-pair subtract gathers (the HANDS17
    pair list decomposes into four affine gathers: root fan-out,
    chain starts, chain middles, chain ends), pair-sum adds, |dot|
    (tensor_scalar abs_max 0), and t = |dot| * rsqrt.
  - Pool (GpSimd, otherwise idle): the two large elementwise
    multiplies (v1*v2 and n1*n2), contiguous bf16 only.
  - PE: ones-vector matmul accumulates t over batch into PSUM.

Per-tile work is software-pipelined in two stages (A: dma/cast/sub/
prod/sq, B: pair-sums/den/rsqrt/abs/t/matmul); B(i) is emitted after
A(i+1) so the in-order engine queues never stall on cross-engine
dependencies. Tile sizes ramp small -> 64 -> small to shorten both the
initial DMA wait and the final drain.
"""
import sys

sys.path.insert(0, "/opt/trn_rl_repo")

from contextlib import ExitStack

import numpy as np

import concourse.bass as bass
import concourse.tile as tile
from concourse import mybir
from concourse.alu_op_type import AluOpType
from concourse.bass_utils import run_bass_kernel_spmd

B, J, DCOORD = 524288, 21, 3
NCORES = 8
P = 128                      # SBUF partitions
F = J * DCOORD               # 63 floats per batch element
NPAIR = 20

f32 = mybir.dt.float32
bf16 = mybir.dt.bfloat16
AF = mybir.ActivationFunctionType


def _split_excess_waits(nc, max_waits: int = 1) -> int:
    """The staged neuronxcc rejects instructions with more than one
    semaphore wait. Same-engine instructions run in order, so excess
    waits move onto preceding NoOps on the same engine."""
    n_split = 0
    for b in nc.m.functions[0].blocks:
        insts = b.instructions
        out = []
        changed = False
        for inst in insts:
            si = getattr(inst, "sync_info", None)
            waits = list(si.on_wait) if si is not None and si.on_wait else []
            if len(waits) > max_waits:
                extra, keep = waits[:-max_waits], waits[-max_waits:]
                while extra:
                    grp, extra = extra[:max_waits], extra[max_waits:]
                    nop = mybir.InstNoOp(
                        name=f"I-waitsplit-{n_split}", engine=inst.engine
                    )
                    nop.sync_info = mybir.SyncInfo(on_wait=grp, on_update=[])
                    out.append(nop)
                    n_split += 1
                inst.sync_info = mybir.SyncInfo(
                    on_wait=keep, on_update=list(si.on_update)
                )
                changed = True
            out.append(inst)
        if changed:
            insts[:] = out
    return n_split


def build_nc(tiles) -> bass.Bass:
    """One core's kernel. `tiles` is the list of per-tile batch counts C
    (batch elements per partition); total batch = P * sum(tiles)."""
    BL = P * sum(tiles)
    nc = bass.Bass()
    x_ext = nc.declare_dram_parameter("jt_uvd_pred", [BL, F], f32, isOutput=False)
    g_ext = nc.declare_dram_parameter("jt_uvd_gt", [BL, F], f32, isOutput=False)
    out_ext = nc.declare_dram_parameter("out", [1, 1], f32, isOutput=True)
    NFMAX = NPAIR * max(tiles)

    with tile.TileContext(nc) as tc, ExitStack() as ctx:
        ins_pool = ctx.enter_context(tc.tile_pool(name="ins", bufs=2))
        mid_pool = ctx.enter_context(tc.tile_pool(name="mid", bufs=2))
        small_pool = ctx.enter_context(tc.tile_pool(name="small", bufs=2))
        const_pool = ctx.enter_context(tc.tile_pool(name="const", bufs=1))
        psum_pool = ctx.enter_context(tc.tile_pool(name="psum", bufs=1, space="PSUM"))

        ones = const_pool.tile([P, 1], bf16)
        nc.vector.memset(ones[:], 1.0)
        # bf16-rounded inputs can collide -> exact-zero bones -> den=0;
        # Rsqrt(den+eps) keeps those pairs at t = 0*huge = 0 instead of NaN
        eps = const_pool.tile([P, 1], f32)
        nc.vector.memset(eps[:], 1e-20)

        # PSUM accumulators for the batch reduction, <=512 f32 per bank.
        # Zeroed up front so variable-size tiles can all accumulate with
        # start=False.
        psums = []
        off = 0
        while off < NFMAX:
            w = min(512, NFMAX - off)
            ps = psum_pool.tile([1, w], f32, name=f"ps{off}", tag=f"ps{off}")
            nc.vector.memset(ps[:], 0.0)
            psums.append((off, w, ps))
            off += w
        last_user = {}
        for i, C in enumerate(tiles):
            for k, (poff, w, ps) in enumerate(psums):
                if NPAIR * C > poff:
                    last_user[k] = i

        state = {}
        b0 = 0

        def emit_a(i):
            nonlocal b0
            C = tiles[i]
            FD = C * F
            rows = P * C
            xv = x_ext[b0 : b0 + rows, :].rearrange("(p c) f -> p (c f)", p=P)
            gv = g_ext[b0 : b0 + rows, :].rearrange("(p c) f -> p (c f)", p=P)
            b0 += rows

            xt = ins_pool.tile([P, FD], f32, tag="xin")
            gt = ins_pool.tile([P, FD], f32, tag="gin")
            nc.sync.dma_start(out=xt[:], in_=xv)
            nc.sync.dma_start(out=gt[:], in_=gv)

            # u[t, k, j, c]: strided f32 -> contiguous bf16, batch innermost
            u = mid_pool.tile([P, 2, 2, J, C], bf16, tag="u")
            xs = xt[:].rearrange("p (c j k) -> p k j c", j=J, k=DCOORD)[:, 0:2]
            gs = gt[:].rearrange("p (c j k) -> p k j c", j=J, k=DCOORD)[:, 0:2]
            nc.scalar.activation(out=u[:, 0], in_=xs, func=AF.Copy)
            nc.vector.tensor_copy(out=u[:, 1], in_=gs)

            # bone vectors dc[tk, q, c]; tk = (tensor, xy) folded: all four
            # gathers share j-strides across tk so each group is one op
            u4 = u[:].rearrange("p t k j c -> p (t k) j c")
            dc = mid_pool.tile([P, 4, NPAIR, C], bf16, tag="dc")
            root = u4[:, :, 0:1, :].broadcast_to([P, 4, 5, C])
            subs = [
                (0, root, u4[:, :, 1:6, :]),
                (5, u4[:, :, 1:6, :], u4[:, :, 6:19:3, :]),
                (10, u4[:, :, 6:19:3, :], u4[:, :, 7:20:3, :]),
                (15, u4[:, :, 7:20:3, :], u4[:, :, 8:21:3, :]),
            ]
            for s0, in0, in1 in subs:
                nc.vector.tensor_sub(out=dc[:, :, s0 : s0 + 5, :], in0=in0, in1=in1)

            # v1*v2 on the (otherwise idle) Pool engine; contiguous halves
            pr = mid_pool.tile([P, 2, NPAIR, C], bf16, tag="pr")
            nc.gpsimd.tensor_mul(
                out=pr[:].rearrange("p k q c -> p (k q c)"),
                in0=dc[:, 0:2].rearrange("p k q c -> p (k q c)"),
                in1=dc[:, 2:4].rearrange("p k q c -> p (k q c)"),
            )
            # squares of all bone coords on ACT
            s = mid_pool.tile([P, 4, NPAIR, C], bf16, tag="s")
            nc.scalar.activation(
                out=s[:].rearrange("p t q c -> p (t q c)"),
                in_=dc[:].rearrange("p t q c -> p (t q c)"),
                func=AF.Square,
            )
            state[i] = (C, pr, s)

        def emit_b(i):
            C, pr, s = state.pop(i)
            NF = NPAIR * C
            # dot = x-part + y-part (contiguous halves)
            dot = small_pool.tile([P, NPAIR, C], bf16, tag="dot")
            nc.vector.tensor_add(out=dot[:], in0=pr[:, 0], in1=pr[:, 1])
            # n[t, q, c] = sum of squared components per tensor
            s4 = s[:].rearrange("p (t k) q c -> p t k q c", t=2)
            n = small_pool.tile([P, 2, NPAIR, C], bf16, tag="n")
            nc.vector.tensor_add(out=n[:], in0=s4[:, :, 0], in1=s4[:, :, 1])
            # den = n1 * n2 on Pool
            den = small_pool.tile([P, NPAIR, C], bf16, tag="den")
            nc.gpsimd.tensor_mul(
                out=den[:].rearrange("p q c -> p (q c)"),
                in0=n[:, 0].rearrange("p q c -> p (q c)"),
                in1=n[:, 1].rearrange("p q c -> p (q c)"),
            )
            # e = 1/sqrt(den) = exp(-0.5*ln(den+eps)) on ACT (Rsqrt is
            # banned in bass for accuracy; Ln/Exp share one table set)
            lg = small_pool.tile([P, NF], bf16, tag="lg")
            nc.scalar.activation(
                out=lg[:],
                in_=den[:].rearrange("p q c -> p (q c)"),
                func=AF.Ln,
                bias=eps[:],
            )
            e = small_pool.tile([P, NF], bf16, tag="e")
            nc.scalar.activation(out=e[:], in_=lg[:], func=AF.Exp, scale=-0.5)
            # t = |dot| * e
            a = small_pool.tile([P, NF], bf16, tag="a")
            nc.scalar.activation(
                out=a[:], in_=dot[:].rearrange("p q c -> p (q c)"), func=AF.Abs
            )
            t = small_pool.tile([P, NF], bf16, tag="t")
            nc.vector.tensor_mul(out=t[:], in0=a[:], in1=e[:])

            for k, (poff, w, ps) in enumerate(psums):
                if NF <= poff:
                    continue
                ww = min(w, NF - poff)
                nc.tensor.matmul(
                    out=ps[:, 0:ww],
                    lhsT=ones[:],
                    rhs=t[:, poff : poff + ww],
                    start=False,
                    stop=(last_user[k] == i),
                    skip_group_check=True,
                )

        for i in range(len(tiles)):
            emit_a(i)
            if i >= 1:
                emit_b(i - 1)
        emit_b(len(tiles) - 1)

        # Tail: reduce each PSUM bank directly (DVE reads PSUM), then the
        # tiny per-bank sums, then DMA the scalar out
        t3 = const_pool.tile([1, len(psums)], f32)
        for k, (poff, w, ps) in enumerate(psums):
            nc.vector.tensor_reduce(
                out=t3[:, k : k + 1],
                in_=ps[:],
                op=mybir.AluOpType.add,
                axis=mybir.AxisListType.X,
            )
        total = const_pool.tile([1, 1], f32)
        nc.vector.tensor_reduce(
            out=total[:], in_=t3[:], op=mybir.AluOpType.add, axis=mybir.AxisListType.X
        )
        nc.sync.dma_start(out=out_ext[:], in_=total[:])

    return nc


_NC_CACHE: dict = {}

DEFAULT_TILES = (16, 32, 48, 64, 64, 64, 64, 64, 48, 32, 16)


def _get_nc(tiles) -> bass.Bass:
    key = tuple(tiles)
    if key not in _NC_CACHE:
        nc = build_nc(list(tiles))
        _split_excess_waits(nc)
        _NC_CACHE[key] = nc
    return _NC_CACHE[key]


def kernel(jt_uvd_pred, jt_uvd_gt, _tiles=DEFAULT_TILES, _trace: bool = False):
    pred = np.ascontiguousarray(np.asarray(jt_uvd_pred), dtype=np.float32)
    gt = np.ascontiguousarray(np.asarray(jt_uvd_gt), dtype=np.float32)
    Btot = pred.shape[0]
    assert pred.shape == (Btot, J, DCOORD) and gt.shape == (Btot, J, DCOORD)
    bl = P * sum(_tiles)
    assert bl * NCORES == Btot, (Btot, _tiles)

    nc = _get_nc(_tiles)
    in_maps = []
    for c in range(NCORES):
        sl = slice(c * bl, (c + 1) * bl)
        in_maps.append(
            {
                "jt_uvd_pred": pred[sl].reshape(bl, F),
                "jt_uvd_gt": gt[sl].reshape(bl, F),
            }
        )
    res = run_bass_kernel_spmd(
        nc, in_maps, core_ids=list(range(NCORES)), trace=_trace
    )
    total = sum(float(res.results[i]["out"][0, 0]) for i in range(NCORES))
    loss = 1.0 - total / (Btot * NPAIR)
    out = np.float32(loss)
    if _trace:
        return out, res
    return out
